# revision 1
# baseline (speedup 1.0000x reference)
"""Trainium2 Bass kernel for nn_Attention (linear attention + 1x1 convs + GroupNorm).

Math (per batch element, reference):
  qkv = W_qkv @ x            (1x1 conv, x: [512, 4096])
  q   = softmax_d(q) * scale ; k = softmax_n(k)
  ctx[h] = k_h @ v_h^T       (tiny [32,32] per head)
  att[h] = ctx[h]^T @ q_h    ([32, 4096])
  y   = W_out @ att + b      ; out = GroupNorm1(y) * gamma + beta

Kernel strategy (data parallel over batch, 2 batch elems per core):
  - q projection in standard layout [128(h d), n]; softmax-over-d denominator D
    via a block-diagonal ones matmul (PE), reciprocal on DVE.
  - k,v projections computed TRANSPOSED ([n, 128]) directly on PE by using the
    x slice as the stationary (lhsT) operand, so the spatial contraction of
    ctx = ek @ v^T is a plain PE accumulation -- no transposes anywhere.
  - k softmax denominator comes free as a ones-column appended to v^T.
  - ctx is masked block-diagonal, so att = ctx2_bd @ eq is one [128,128,512]
    matmul per slice; the q softmax division folds into the PSUM->SBUF move.
  - GroupNorm folds entirely into the output projection epilogue:
      out = (y - mu) * rstd * gamma + beta = A[c] * (Wout@att)[c,n] + B[c]
    with stats from att only: S1 = <colsum_W, r> + 4096*sum(b),
    S2 = ||L^T att||_F^2 + 2<W^T b, r> + 4096*sum(b^2), G = W^T W = L L^T.
    A/B ride the ACT PSUM->SBUF copy as per-partition scale/bias.
  - all matmuls run as float32r (TF32-like, full speed at N>=256, ~1e-4 rel).
"""

import numpy as np

B, C, HGT, WID = 16, 512, 64, 64
NSP = HGT * WID            # 4096 spatial
HEADS, DH = 4, 32
HID = HEADS * DH           # 128
NCORES = 8
BPC = B // NCORES          # 2 batch elems per core
SCALE = DH ** -0.5
EPS = 1e-5
SLICE = 512                # spatial slice for q/D/out2/final matmuls
NSL = NSP // SLICE         # 8
KC = C // 128              # 4 contraction chunks
MC = C // 128              # 4 output-channel chunks
VTW = 132                  # v^T tile width: 128 v cols + 4 ones cols

# pool buffer counts (tunable via env for experiments)
import os as _os
def _env(k, d):
    return int(_os.environ.get(k, d))
BUF_X = _env("KBUF_X", 3)
BUF_EQ = _env("KBUF_EQ", 2)
BUF_EKT = _env("KBUF_EKT", 2)
BUF_VTA = _env("KBUF_VTA", 2)
BUF_RD = _env("KBUF_RD", 1)
BUF_ATT = _env("KBUF_ATT", 1)
BUF_Y = _env("KBUF_Y", 4)
BUF_PSUM = _env("KBUF_PSUM", 4)
BUF_PSCD = _env("KBUF_PSCD", 3)
BUF_PSY = _env("KBUF_PSY", 0)
BUF_PSCTX = _env("KBUF_PSCTX", 1)
YSPLIT = _env("KYSPLIT", 1)  # 1: alternate output-copy between ACT and DVE
CTX_BF16 = _env("KCTX_BF16", 0)  # 1: ekT/vT + ctx matmul in bf16
KSTORE = _env("KSTORE", 0)  # output store queue: 0=sync 1=gpsimd 2=scalar
KPIPE = _env("KPIPE", 1)  # emit phase E(b-1) after phase A(b); best steady-state
KWARM = _env("KWARM", 0)  # N junk matmuls to warm the PE clock at start
KZIP = _env("KZIP", 1)  # Square writes back into PSUM in place

_CACHE = {}


def _build_nc(repeat=1):
    import concourse.bass as bass
    import concourse.mybir as mybir
    import concourse.tile as tile
    from concourse import bacc

    f32 = mybir.dt.float32
    f32r = mybir.dt.float32r
    bf16 = mybir.dt.bfloat16
    ctxdt = bf16 if CTX_BF16 else f32r
    AF = mybir.ActivationFunctionType
    OP = mybir.AluOpType
    AX = mybir.AxisListType

    nc = bacc.Bacc("TRN2", target_bir_lowering=False, debug=False)

    x_d = nc.dram_tensor("x", [BPC, C, NSP], f32r, kind="ExternalInput")
    wq_d = nc.dram_tensor("wq_t", [C, HID], f32r, kind="ExternalInput")
    wkv_d = nc.dram_tensor("wkv_t", [C, 2 * HID], f32r, kind="ExternalInput")
    wout_d = nc.dram_tensor("wout_t", [HID, C], f32r, kind="ExternalInput")
    bones_d = nc.dram_tensor("b_ones", [HID, HID], f32r, kind="ExternalInput")
    mask_d = nc.dram_tensor("mask_scale", [HID, HID], f32, kind="ExternalInput")
    lmat_d = nc.dram_tensor("lmat", [HID, HID], f32r, kind="ExternalInput")
    u2_d = nc.dram_tensor("u2", [HID, 4], f32r, kind="ExternalInput")
    ones_d = nc.dram_tensor("ones_col", [HID, 4], f32r, kind="ExternalInput")
    onesr_d = nc.dram_tensor("ones_row", [1, HID], f32r, kind="ExternalInput")
    vones_d = nc.dram_tensor("vones", [128, 32, VTW - 128], ctxdt, kind="ExternalInput")
    g4_d = nc.dram_tensor("gamma4", [128, MC], f32, kind="ExternalInput")
    gb4_d = nc.dram_tensor("gb4", [128, MC], f32, kind="ExternalInput")
    be4_d = nc.dram_tensor("beta4", [128, MC], f32, kind="ExternalInput")
    sc_d = nc.dram_tensor("sc", [1, 8], f32, kind="ExternalInput")
    out_d = nc.dram_tensor("out", [BPC, C, NSP], f32, kind="ExternalOutput")

    def r(ap):
        return ap.bitcast(f32r)

    with tile.TileContext(nc) as tc:
        with (
            tc.tile_pool(name="consts", bufs=1) as consts,
            tc.tile_pool(name="xp", bufs=BUF_X) as xp,
            tc.tile_pool(name="eqp", bufs=BUF_EQ) as eqp,
            tc.tile_pool(name="ektp", bufs=BUF_EKT) as ektp,
            tc.tile_pool(name="vtap", bufs=BUF_VTA) as vtap,
            tc.tile_pool(name="rdp", bufs=BUF_RD) as rdp,
            tc.tile_pool(name="attp", bufs=BUF_ATT) as attp,
            tc.tile_pool(name="yp", bufs=BUF_Y) as yp,
            tc.tile_pool(name="zp", bufs=2) as zp,
            tc.tile_pool(name="smalls", bufs=4) as smalls,
            tc.tile_pool(name="stp", bufs=2) as stp,
            tc.tile_pool(name="pp", bufs=BUF_PSUM, space="PSUM") as pp,
            tc.tile_pool(name="ppcd", bufs=BUF_PSCD, space="PSUM") as ppcd,
            tc.tile_pool(name="ppy", bufs=max(BUF_PSY, 1), space="PSUM") as ppy,
            tc.tile_pool(name="ppctx", bufs=BUF_PSCTX, space="PSUM") as ppctx,
        ):
            # --- constants: tiles up front; DMAs staged by first use ---
            wq_sb = consts.tile([128, KC, HID], f32r)
            wkv_sb = consts.tile([128, KC, 2 * HID], f32r)
            wout_sb = consts.tile([128, C], f32r)
            bones_sb = consts.tile([128, HID], f32r)
            mask_sb = consts.tile([128, HID], f32)
            lmat_sb = consts.tile([128, HID], f32r)
            u2_sb = consts.tile([128, 4], f32r)
            ones_sb = consts.tile([128, 4], f32r)
            onesr_sb = consts.tile([1, HID], f32r)
            g4_sb = consts.tile([128, MC], f32)
            gb4_sb = consts.tile([128, MC], f32)
            be4_sb = consts.tile([128, MC], f32)
            sc_sb = consts.tile([1, 8], f32)

            if KWARM:
                wtile = consts.tile([128, 4], f32r, name="wtile")
                nc.sync.dma_start(out=wtile, in_=ones_d.ap())
                psw = pp.tile([128, SLICE], f32, tag="ps", name="psw")
                for _w in range(KWARM):
                    nc.tensor.matmul(
                        psw[0:4, 0:4], wtile[:, :], wtile[:, :],
                        start=True, stop=True,
                    )

            # needed by phase A (q/kv projections + D matmul)
            nc.gpsimd.dma_start(
                out=wq_sb, in_=wq_d.ap().rearrange("(cc p) m -> p cc m", p=128)
            )
            nc.gpsimd.dma_start(
                out=wkv_sb, in_=wkv_d.ap().rearrange("(cc p) m -> p cc m", p=128)
            )
            nc.gpsimd.dma_start(out=bones_sb, in_=bones_d.ap())

            # prime the DMA pipe: first two x slices of the first batch
            pre_xs = {}
            _xa0 = x_d.ap()[0].rearrange("(cc p) n -> p cc n", p=128)
            for _j in range(2):
                pxs = xp.tile([128, KC, SLICE], f32r, tag="xs", name="pxs")
                nc.sync.dma_start(
                    out=pxs, in_=_xa0[:, :, _j * SLICE : (_j + 1) * SLICE]
                )
                pre_xs[(0, _j)] = pxs

            def mid_consts():
                # needed by phases B-D
                nc.gpsimd.dma_start(out=mask_sb, in_=mask_d.ap())
                nc.gpsimd.dma_start(out=lmat_sb, in_=lmat_d.ap())
                nc.gpsimd.dma_start(out=u2_sb, in_=u2_d.ap())
                nc.gpsimd.dma_start(out=ones_sb, in_=ones_d.ap())
                nc.gpsimd.dma_start(out=onesr_sb, in_=onesr_d.ap())
                nc.gpsimd.dma_start(out=sc_sb, in_=sc_d.ap())

            def late_consts():
                # needed by phase E
                nc.gpsimd.dma_start(out=wout_sb, in_=wout_d.ap())
                nc.gpsimd.dma_start(out=g4_sb, in_=g4_d.ap())
                nc.gpsimd.dma_start(out=gb4_sb, in_=gb4_d.ap())
                nc.gpsimd.dma_start(out=be4_sb, in_=be4_d.ap())

            batch_seq = [bi for _ in range(repeat) for bi in range(BPC)]

            def phase_a_init(b, i_b=-1):
                xa = x_d.ap()[b].rearrange("(cc p) n -> p cc n", p=128)
                eq = eqp.tile([128, NSP], f32r, name="eq")
                ekt = ektp.tile([128, 32, 128], ctxdt, name="ekt")
                vta = vtap.tile([128, 32, VTW], ctxdt, name="vta")
                nc.gpsimd.dma_start(out=vta[:, :, 128:VTW], in_=vones_d.ap())
                recipd = rdp.tile([128, NSP], f32, name="recipd")
                return {"xa": xa, "eq": eq, "ekt": ekt, "vta": vta,
                        "recipd": recipd, "i_b": i_b}

            def phase_a_slice(S, j):
                xa, eq, ekt, vta, recipd = (
                    S["xa"], S["eq"], S["ekt"], S["vta"], S["recipd"]
                )
                if True:
                    sl = slice(j * SLICE, (j + 1) * SLICE)
                    if (S["i_b"], j) in pre_xs:
                        xs = pre_xs.pop((S["i_b"], j))
                    else:
                        xs = xp.tile([128, KC, SLICE], f32r, tag="xs", name="xs")
                        nc.sync.dma_start(out=xs, in_=xa[:, :, sl])

                    # q projection (standard layout) + exp
                    psq = pp.tile([128, SLICE], f32, tag="ps", name="psq")
                    for cc in range(KC):
                        nc.tensor.matmul(
                            psq[:, :],
                            r(wq_sb[:, cc, :]),
                            r(xs[:, cc, :]),
                            start=(cc == 0),
                            stop=(cc == KC - 1),
                        )
                    nc.scalar.activation(out=eq[:, sl], in_=psq[:, :], func=AF.Exp)

                    # softmax-over-d denominator, broadcast within head + recip
                    psd = pp.tile([128, SLICE], f32, tag="ps", name="psd")
                    nc.tensor.matmul(
                        psd[:, :], r(bones_sb[:, :]), r(eq[:, sl]), start=True, stop=True
                    )
                    nc.vector.reciprocal_approx_fast(out=recipd[:, sl], in_=psd[:, :])

                    # k,v transposed projections: x sub-slices as stationary operand
                    for h in range(2):
                        pskv = pp.tile([128, SLICE], f32, tag="ps", name="pskv")
                        for s_ in range(2):
                            isub = 2 * h + s_
                            for cc in range(KC):
                                nc.tensor.matmul(
                                    pskv[:, s_ * 256 : (s_ + 1) * 256],
                                    r(xs[:, cc, isub * 128 : (isub + 1) * 128]),
                                    r(wkv_sb[:, cc, :]),
                                    start=(cc == 0),
                                    stop=(cc == KC - 1),
                                )
                        kvv = pskv[:, :].rearrange("p (s o) -> p s o", s=2)
                        a0 = 4 * j + 2 * h
                        nc.scalar.activation(
                            out=ekt[:, a0 : a0 + 2, :], in_=kvv[:, :, 0:128], func=AF.Exp
                        )
                        nc.vector.tensor_copy(
                            out=vta[:, a0 : a0 + 2, 0:128], in_=kvv[:, :, 128:256]
                        )

            def phase_bcd(b, S):
                eq, ekt, vta, recipd = S["eq"], S["ekt"], S["vta"], S["recipd"]
                att = attp.tile([128, NSP], f32r, name="att")
                racc = smalls.tile([128, NSL], f32, name="racc")
                zacc = smalls.tile([128, NSL], f32, name="zacc")

                # context (accumulate over all 32 spatial chunks)
                psctx = ppctx.tile([128, SLICE], f32, name="psctx")
                for i in range(32):
                    nc.tensor.matmul(
                        psctx[:, 0:VTW],
                        ekt[:, i, :] if CTX_BF16 else r(ekt[:, i, :]),
                        vta[:, i, :] if CTX_BF16 else r(vta[:, i, :]),
                        start=(i == 0),
                        stop=(i == 31),
                    )
                inv_sk = smalls.tile([128, 1], f32, name="inv_sk")
                nc.vector.reciprocal_approx_fast(out=inv_sk, in_=psctx[:, 128:129])
                ctx2 = smalls.tile([128, HID], f32r, name="ctx2")
                nc.vector.scalar_tensor_tensor(
                    out=ctx2,
                    in0=psctx[:, 0:HID],
                    scalar=inv_sk,
                    in1=mask_sb[:, :],
                    op0=OP.mult,
                    op1=OP.mult,
                )

                # att = (ctx2_bd @ eq) * recipD; z = L^T att; accumulate stats
                for j in range(NSL):
                    sl = slice(j * SLICE, (j + 1) * SLICE)
                    pso = ppcd.tile([128, SLICE], f32, tag="pscd", name="pso")
                    nc.tensor.matmul(
                        pso[:, :], r(ctx2[:, :]), r(eq[:, sl]), start=True, stop=True
                    )
                    nc.vector.scalar_tensor_tensor(
                        out=att[:, sl],
                        in0=pso[:, :],
                        scalar=1.0,
                        in1=recipd[:, sl],
                        op0=OP.bypass,
                        op1=OP.mult,
                        accum_out=racc[:, j : j + 1],
                    )
                    psz = ppcd.tile([128, SLICE], f32, tag="pscd", name="psz")
                    nc.tensor.matmul(
                        psz[:, :], r(lmat_sb[:, :]), r(att[:, sl]), start=True, stop=True
                    )
                    if KZIP:
                        nc.scalar.activation(
                            out=psz[:, :], in_=psz[:, :], func=AF.Square,
                            accum_out=zacc[:, j : j + 1],
                        )
                    else:
                        zs = zp.tile([128, SLICE], f32, name="zs")
                        nc.scalar.activation(
                            out=zs, in_=psz[:, :], func=AF.Square,
                            accum_out=zacc[:, j : j + 1],
                        )

                r_sb = smalls.tile([128, 1], f32, name="r_sb")
                nc.vector.tensor_reduce(out=r_sb, in_=racc, axis=AX.X, op=OP.add)
                zred = smalls.tile([128, 1], f32, name="zred")
                nc.vector.tensor_reduce(out=zred, in_=zacc, axis=AX.X, op=OP.add)
                # duplicate into even-width f32r stationary operands
                rr = smalls.tile([128, 2], f32r, name="rr")
                nc.vector.tensor_copy(out=rr[:, 0:1], in_=r_sb)
                nc.vector.tensor_copy(out=rr[:, 1:2], in_=r_sb)
                zz = smalls.tile([128, 2], f32r, name="zz")
                nc.vector.tensor_copy(out=zz[:, 0:1], in_=zred)
                nc.vector.tensor_copy(out=zz[:, 1:2], in_=zred)

                # partition-0 dots: psA[0, 0]=<colsum_W,r>, psA[0, 1]=<W^T b, r>
                psA = ppcd.tile([128, SLICE], f32, tag="pscd", name="psA")
                nc.tensor.matmul(
                    psA[0:2, 0:4], r(rr[:, :]), r(u2_sb[:, :]), start=True, stop=True
                )
                # psB[0, 0] = sum_p zred = ||z||^2 total
                psB = ppcd.tile([128, SLICE], f32, tag="pscd", name="psB")
                nc.tensor.matmul(
                    psB[0:2, 0:4], r(zz[:, :]), r(ones_sb[:, :]), start=True, stop=True
                )

                # scalar chain on partition 0 (fused):
                st = stp.tile([1, 16], f32, name="st")
                inv_n = 1.0 / (C * NSP)
                nc.vector.scalar_tensor_tensor(
                    out=st[:, 2:3], in0=psA[0:1, 0:1], scalar=inv_n,
                    in1=sc_sb[:, 4:5], op0=OP.mult, op1=OP.add,
                )
                nc.vector.scalar_tensor_tensor(
                    out=st[:, 0:1], in0=psA[0:1, 1:2], scalar=2.0 * inv_n,
                    in1=sc_sb[:, 5:6], op0=OP.mult, op1=OP.add,
                )
                nc.vector.scalar_tensor_tensor(
                    out=st[:, 1:2], in0=psB[0:1, 0:1], scalar=inv_n,
                    in1=st[:, 0:1], op0=OP.mult, op1=OP.add,
                )
                nc.vector.tensor_mul(out=st[:, 3:4], in0=st[:, 2:3], in1=st[:, 2:3])
                nc.vector.tensor_sub(out=st[:, 4:5], in0=st[:, 1:2], in1=st[:, 3:4])
                # rstd = rsqrt(se), DVE-only: bit-trick seed + 3 Newton steps
                # (avoids Ln/Sqrt ACT table-set switches; se > 0 always)
                i32 = mybir.dt.int32
                nc.vector.tensor_scalar(
                    out=st[:, 6:7].bitcast(i32), in0=st[:, 4:5].bitcast(i32),
                    scalar1=1, scalar2=None, op0=OP.arith_shift_right,
                )
                nc.vector.tensor_scalar(
                    out=st[:, 6:7].bitcast(i32), in0=st[:, 6:7].bitcast(i32),
                    scalar1=-1, scalar2=0x5F3759DF,
                    op0=OP.mult, op1=OP.add,
                )
                for it in range(3):
                    dst = st[:, 8:9] if it == 2 else st[:, 6:7]
                    nc.vector.tensor_mul(out=st[:, 10:11], in0=st[:, 6:7], in1=st[:, 6:7])
                    nc.vector.tensor_mul(out=st[:, 10:11], in0=st[:, 10:11], in1=st[:, 4:5])
                    nc.vector.tensor_scalar(
                        out=st[:, 10:11], in0=st[:, 10:11],
                        scalar1=-0.5, scalar2=1.5, op0=OP.mult, op1=OP.add,
                    )
                    nc.vector.tensor_mul(out=dst, in0=st[:, 6:7], in1=st[:, 10:11])
                nc.vector.tensor_mul(out=st[:, 9:10], in0=st[:, 2:3], in1=st[:, 8:9])

                # broadcast (rstd, mu*rstd) across partitions via a K=1 matmul
                stb = stp.tile([1, 2], f32r, name="stb")
                nc.vector.tensor_copy(out=stb, in_=st[:, 8:10])
                psb = ppcd.tile([128, SLICE], f32, tag="pscd", name="psb")
                nc.tensor.matmul(
                    psb[:, 0:2], r(onesr_sb[:, :]), r(stb[:, :]), start=True, stop=True
                )
                rb = smalls.tile([128, 2], f32, name="rb")
                nc.vector.tensor_copy(out=rb, in_=psb[:, 0:2])

                # A = gamma*rstd ; Bc = gb*rstd - gamma*(mu*rstd) + beta
                a4 = smalls.tile([128, MC], f32, name="a4")
                nc.vector.tensor_scalar_mul(out=a4, in0=g4_sb[:, :], scalar1=rb[:, 0:1])
                b4 = smalls.tile([128, MC], f32, name="b4")
                nc.vector.tensor_scalar_mul(out=b4, in0=gb4_sb[:, :], scalar1=rb[:, 0:1])
                t4 = smalls.tile([128, MC], f32, name="t4")
                nc.vector.tensor_scalar_mul(out=t4, in0=g4_sb[:, :], scalar1=rb[:, 1:2])
                nc.vector.tensor_sub(out=b4, in0=b4, in1=t4)
                nc.vector.tensor_add(out=b4, in0=b4, in1=be4_sb[:, :])
                S["att"] = att
                S["a4"] = a4
                S["b4"] = b4

            def phase_e(b, S):
                att, a4, b4 = S["att"], S["a4"], S["b4"]
                oa = out_d.ap()[b].rearrange("(mm p) n -> p mm n", p=128)
                for j in range(NSL):
                    sl = slice(j * SLICE, (j + 1) * SLICE)
                    ys4 = yp.tile([128, MC, SLICE], f32, name="ys4")
                    for m in range(MC):
                        psy_pool, psy_tag = (ppy, "psy") if BUF_PSY else (ppcd, "pscd")
                        psy = psy_pool.tile([128, SLICE], f32, tag=psy_tag, name="psy")
                        nc.tensor.matmul(
                            psy[:, :],
                            r(wout_sb[:, m * 128 : (m + 1) * 128]),
                            r(att[:, sl]),
                            start=True,
                            stop=True,
                        )
                        if YSPLIT and (m % 2 == 1):
                            nc.vector.tensor_scalar(
                                out=ys4[:, m, :], in0=psy[:, :],
                                scalar1=a4[:, m : m + 1], scalar2=b4[:, m : m + 1],
                                op0=OP.mult, op1=OP.add,
                            )
                        else:
                            nc.scalar.activation(
                                out=ys4[:, m, :], in_=psy[:, :], func=AF.Identity,
                                bias=b4[:, m : m + 1], scale=a4[:, m : m + 1],
                            )
                    store_eng = [nc.sync, nc.gpsimd, nc.scalar][KSTORE]
                    store_eng.dma_start(out=oa[:, :, sl], in_=ys4)

            def phase_a(b, i_b=-1):
                S = phase_a_init(b, i_b)
                for j in range(NSL):
                    phase_a_slice(S, j)
                return S

            if KPIPE == 2:
                # interleave the BPC batches of each repeat block slice-by-slice
                for rep in range(repeat):
                    bs = batch_seq[rep * BPC : (rep + 1) * BPC]
                    Ss = [phase_a_init(b, rep * BPC + k) for k, b in enumerate(bs)]
                    mid_consts()
                    for j in range(NSL):
                        for S in Ss:
                            phase_a_slice(S, j)
                    late_consts()
                    for b, S in zip(bs, Ss):
                        phase_bcd(b, S)
                    for b, S in zip(bs, Ss):
                        phase_e(b, S)
            elif KPIPE in (1, 3, 4, 5, 6):
                states = []
                mid_consts()
                late_consts()
                for i_b, b in enumerate(batch_seq):
                    S = phase_a_init(b, i_b)
                    states.append(S)
                    cut = {3: 3, 4: 5, 5: 4, 6: 6}.get(KPIPE)
                    for j in range(NSL):
                        phase_a_slice(S, j)
                        if j == cut and i_b > 0:
                            phase_e(batch_seq[i_b - 1], states[i_b - 1])
                    if KPIPE == 1 and i_b > 0:
                        phase_e(batch_seq[i_b - 1], states[i_b - 1])
                    phase_bcd(b, S)
                phase_e(batch_seq[-1], states[-1])
            else:
                mid_consts()
                late_consts()
                for i_b, b in enumerate(batch_seq):
                    S = phase_a(b, i_b)
                    phase_bcd(b, S)
                    phase_e(b, S)

    nc.compile()
    return nc


def _host_consts(W_qkv, W_out, b_out, gamma, beta):
    W_qkv = np.asarray(W_qkv, np.float32)
    W_out = np.asarray(W_out, np.float32)
    b_out = np.asarray(b_out, np.float32)
    gamma = np.asarray(gamma, np.float32)
    beta = np.asarray(beta, np.float32)

    c = {}
    c["wq_t"] = np.ascontiguousarray(W_qkv[:HID].T)                 # [512, 128]
    c["wkv_t"] = np.ascontiguousarray(W_qkv[HID : 3 * HID].T)       # [512, 256]
    c["wout_t"] = np.ascontiguousarray(W_out.T)                     # [128, 512]
    blk = np.kron(np.eye(HEADS, dtype=np.float32), np.ones((DH, DH), np.float32))
    c["b_ones"] = blk                                               # [128, 128]
    c["mask_scale"] = (blk * SCALE).astype(np.float32)
    G = (W_out.astype(np.float64).T @ W_out.astype(np.float64))
    try:
        L = np.linalg.cholesky(G)
    except np.linalg.LinAlgError:
        w_ev, V = np.linalg.eigh(G)
        L = V @ np.diag(np.sqrt(np.clip(w_ev, 0.0, None)))
    c["lmat"] = L.astype(np.float32)                                # [128, 128]
    u2 = np.zeros((HID, 4), np.float32)
    u2[:, 0] = W_out.sum(axis=0)
    u2[:, 1] = W_out.T @ b_out
    c["u2"] = u2                                                    # [128, 4]
    c["ones_col"] = np.ones((HID, 4), np.float32)
    c["ones_row"] = np.ones((1, HID), np.float32)
    if CTX_BF16:
        import ml_dtypes
        c["vones"] = np.ones((128, 32, VTW - 128), ml_dtypes.bfloat16)
    else:
        c["vones"] = np.ones((128, 32, VTW - 128), np.float32)
    c["gamma4"] = np.ascontiguousarray(gamma.reshape(MC, 128).T)
    c["gb4"] = np.ascontiguousarray((gamma * b_out).reshape(MC, 128).T)
    c["beta4"] = np.ascontiguousarray(beta.reshape(MC, 128).T)
    ntot = float(C) * float(NSP)
    sc = np.zeros((1, 8), np.float32)
    c1 = NSP * float(b_out.astype(np.float64).sum())
    c2 = NSP * float((b_out.astype(np.float64) ** 2).sum())
    sc[0, 0] = c1
    sc[0, 1] = c2
    sc[0, 2] = EPS
    sc[0, 3] = 1.0 / ntot
    sc[0, 4] = c1 / ntot                         # C1*invN
    sc[0, 5] = c2 / ntot + EPS                   # C2*invN + eps
    c["sc"] = sc
    return c


def kernel(x, W_qkv, W_out, b_out, gamma, beta):
    from concourse.bass_utils import run_bass_kernel_spmd

    x = np.asarray(x, np.float32)
    assert x.shape == (B, C, HGT, WID)
    if "nc" not in _CACHE:
        _CACHE["nc"] = _build_nc()
    nc = _CACHE["nc"]

    consts = _host_consts(W_qkv, W_out, b_out, gamma, beta)
    xr = x.reshape(B, C, NSP)
    in_maps = []
    for ci in range(NCORES):
        m = {"x": np.ascontiguousarray(xr[ci * BPC : (ci + 1) * BPC])}
        m.update(consts)
        in_maps.append(m)

    res = run_bass_kernel_spmd(nc, in_maps, core_ids=list(range(NCORES)))
    out = np.concatenate([r_["out"] for r_ in res.results], axis=0)
    return np.ascontiguousarray(out.reshape(B, C, HGT, WID).astype(np.float32))



# revision 5
# speedup vs baseline: 2.7856x; 2.7856x over previous
"""Trainium2 Bass kernel for nn_Attention (linear attention + 1x1 convs + GroupNorm).

Math (per batch element, reference):
  qkv = W_qkv @ x            (1x1 conv, x: [512, 4096])
  q   = softmax_d(q) * scale ; k = softmax_n(k)
  ctx[h] = k_h @ v_h^T       (tiny [32,32] per head)
  att[h] = ctx[h]^T @ q_h    ([32, 4096])
  y   = W_out @ att + b      ; out = GroupNorm1(y) * gamma + beta

Kernel strategy (data parallel over batch, 2 batch elems per core):
  - q projection in standard layout [128(h d), n]; softmax-over-d denominator D
    via a block-diagonal ones matmul (PE), reciprocal on DVE.
  - k,v projections computed TRANSPOSED ([n, 128]) directly on PE by using the
    x slice as the stationary (lhsT) operand, so the spatial contraction of
    ctx = ek @ v^T is a plain PE accumulation -- no transposes anywhere.
  - k softmax denominator comes free as a ones-column appended to v^T.
  - ctx is masked block-diagonal, so att = ctx2_bd @ eq is one [128,128,512]
    matmul per slice; the q softmax division folds into the PSUM->SBUF move.
  - GroupNorm folds entirely into the output projection epilogue:
      out = (y - mu) * rstd * gamma + beta = A[c] * (Wout@att)[c,n] + B[c]
    with stats from att only: S1 = <colsum_W, r> + 4096*sum(b),
    S2 = ||L^T att||_F^2 + 2<W^T b, r> + 4096*sum(b^2), G = W^T W = L L^T.
    A/B ride the ACT PSUM->SBUF copy as per-partition scale/bias.
  - all matmuls run as float32r (TF32-like, full speed at N>=256, ~1e-4 rel).
"""

import numpy as np

B, C, HGT, WID = 16, 512, 64, 64
NSP = HGT * WID            # 4096 spatial
HEADS, DH = 4, 32
HID = HEADS * DH           # 128
NCORES = 8
BPC = B // NCORES          # 2 batch elems per core
SCALE = DH ** -0.5
EPS = 1e-5
SLICE = 512                # spatial slice for q/D/out2/final matmuls
NSL = NSP // SLICE         # 8
KC = C // 128              # 4 contraction chunks
MC = C // 128              # 4 output-channel chunks
VTW = 132                  # v^T tile width: 128 v cols + 4 ones cols

# pool buffer counts (tunable via env for experiments)
import os as _os
def _env(k, d):
    return int(_os.environ.get(k, d))
BUF_X = _env("KBUF_X", 3)
BUF_EQ = _env("KBUF_EQ", 2)
BUF_EKT = _env("KBUF_EKT", 2)
BUF_VTA = _env("KBUF_VTA", 2)
BUF_RD = _env("KBUF_RD", 1)
BUF_ATT = _env("KBUF_ATT", 1)
BUF_Y = _env("KBUF_Y", 4)
BUF_PSUM = _env("KBUF_PSUM", 4)
BUF_PSCD = _env("KBUF_PSCD", 3)
BUF_PSY = _env("KBUF_PSY", 0)
BUF_PSCTX = _env("KBUF_PSCTX", 1)
YSPLIT = _env("KYSPLIT", 1)  # 1: alternate output-copy between ACT and DVE
CTX_BF16 = _env("KCTX_BF16", 0)  # 1: ekT/vT + ctx matmul in bf16
KSTORE = _env("KSTORE", 0)  # output store queue: 0=sync 1=gpsimd 2=scalar
KPIPE = _env("KPIPE", 1)  # emit phase E(b-1) after phase A(b); best steady-state
KWARM = _env("KWARM", 0)  # N junk matmuls to warm the PE clock at start
KZIP = _env("KZIP", 1)  # Square writes back into PSUM in place

_CACHE = {}


def _build_nc(repeat=1, ctx_bf16=None):
    import concourse.bass as bass
    import concourse.mybir as mybir
    import concourse.tile as tile
    from concourse import bacc

    if ctx_bf16 is None:
        ctx_bf16 = CTX_BF16
    f32 = mybir.dt.float32
    f32r = mybir.dt.float32r
    bf16 = mybir.dt.bfloat16
    ctxdt = bf16 if ctx_bf16 else f32r
    AF = mybir.ActivationFunctionType
    OP = mybir.AluOpType
    AX = mybir.AxisListType

    nc = bacc.Bacc("TRN2", target_bir_lowering=False, debug=False)

    x_d = nc.dram_tensor("x", [BPC, C, NSP], f32r, kind="ExternalInput")
    wq_d = nc.dram_tensor("wq_t", [C, HID], f32r, kind="ExternalInput")
    wkv_d = nc.dram_tensor("wkv_t", [C, 2 * HID], f32r, kind="ExternalInput")
    wout_d = nc.dram_tensor("wout_t", [HID, C], f32r, kind="ExternalInput")
    bones_d = nc.dram_tensor("b_ones", [HID, HID], f32r, kind="ExternalInput")
    mask_d = nc.dram_tensor("mask_scale", [HID, HID], f32, kind="ExternalInput")
    lmat_d = nc.dram_tensor("lmat", [HID, HID], f32r, kind="ExternalInput")
    u2_d = nc.dram_tensor("u2", [HID, 4], f32r, kind="ExternalInput")
    ones_d = nc.dram_tensor("ones_col", [HID, 4], f32r, kind="ExternalInput")
    onesr_d = nc.dram_tensor("ones_row", [1, HID], f32r, kind="ExternalInput")
    vones_d = nc.dram_tensor("vones", [128, 32, VTW - 128], ctxdt, kind="ExternalInput")
    g4_d = nc.dram_tensor("gamma4", [128, MC], f32, kind="ExternalInput")
    gb4_d = nc.dram_tensor("gb4", [128, MC], f32, kind="ExternalInput")
    be4_d = nc.dram_tensor("beta4", [128, MC], f32, kind="ExternalInput")
    sc_d = nc.dram_tensor("sc", [1, 8], f32, kind="ExternalInput")
    out_d = nc.dram_tensor("out", [BPC, C, NSP], f32, kind="ExternalOutput")

    def r(ap):
        return ap.bitcast(f32r)

    with tile.TileContext(nc) as tc:
        with (
            tc.tile_pool(name="consts", bufs=1) as consts,
            tc.tile_pool(name="xp", bufs=BUF_X) as xp,
            tc.tile_pool(name="eqp", bufs=BUF_EQ) as eqp,
            tc.tile_pool(name="ektp", bufs=BUF_EKT) as ektp,
            tc.tile_pool(name="vtap", bufs=BUF_VTA) as vtap,
            tc.tile_pool(name="rdp", bufs=BUF_RD) as rdp,
            tc.tile_pool(name="attp", bufs=BUF_ATT) as attp,
            tc.tile_pool(name="yp", bufs=BUF_Y) as yp,
            tc.tile_pool(name="zp", bufs=2) as zp,
            tc.tile_pool(name="smalls", bufs=4) as smalls,
            tc.tile_pool(name="stp", bufs=2) as stp,
            tc.tile_pool(name="pp", bufs=BUF_PSUM, space="PSUM") as pp,
            tc.tile_pool(name="ppcd", bufs=BUF_PSCD, space="PSUM") as ppcd,
            tc.tile_pool(name="ppy", bufs=max(BUF_PSY, 1), space="PSUM") as ppy,
            tc.tile_pool(name="ppctx", bufs=BUF_PSCTX, space="PSUM") as ppctx,
        ):
            # --- constants: tiles up front; DMAs staged by first use ---
            wq_sb = consts.tile([128, KC, HID], f32r)
            wkv_sb = consts.tile([128, KC, 2 * HID], f32r)
            wout_sb = consts.tile([128, C], f32r)
            bones_sb = consts.tile([128, HID], f32r)
            mask_sb = consts.tile([128, HID], f32)
            lmat_sb = consts.tile([128, HID], f32r)
            u2_sb = consts.tile([128, 4], f32r)
            ones_sb = consts.tile([128, 4], f32r)
            onesr_sb = consts.tile([1, HID], f32r)
            g4_sb = consts.tile([128, MC], f32)
            gb4_sb = consts.tile([128, MC], f32)
            be4_sb = consts.tile([128, MC], f32)
            sc_sb = consts.tile([1, 8], f32)

            if KWARM:
                wtile = consts.tile([128, 4], f32r, name="wtile")
                nc.sync.dma_start(out=wtile, in_=ones_d.ap())
                psw = pp.tile([128, SLICE], f32, tag="ps", name="psw")
                for _w in range(KWARM):
                    nc.tensor.matmul(
                        psw[0:4, 0:4], wtile[:, :], wtile[:, :],
                        start=True, stop=True,
                    )

            # needed by phase A (q/kv projections + D matmul)
            nc.gpsimd.dma_start(
                out=wq_sb, in_=wq_d.ap().rearrange("(cc p) m -> p cc m", p=128)
            )
            nc.gpsimd.dma_start(
                out=wkv_sb, in_=wkv_d.ap().rearrange("(cc p) m -> p cc m", p=128)
            )
            nc.gpsimd.dma_start(out=bones_sb, in_=bones_d.ap())

            # prime the DMA pipe: first two x slices of the first batch
            pre_xs = {}
            _xa0 = x_d.ap()[0].rearrange("(cc p) n -> p cc n", p=128)
            for _j in range(2):
                pxs = xp.tile([128, KC, SLICE], f32r, tag="xs", name="pxs")
                nc.sync.dma_start(
                    out=pxs, in_=_xa0[:, :, _j * SLICE : (_j + 1) * SLICE]
                )
                pre_xs[(0, _j)] = pxs

            def mid_consts():
                # needed by phases B-D
                nc.gpsimd.dma_start(out=mask_sb, in_=mask_d.ap())
                nc.gpsimd.dma_start(out=lmat_sb, in_=lmat_d.ap())
                nc.gpsimd.dma_start(out=u2_sb, in_=u2_d.ap())
                nc.gpsimd.dma_start(out=ones_sb, in_=ones_d.ap())
                nc.gpsimd.dma_start(out=onesr_sb, in_=onesr_d.ap())
                nc.gpsimd.dma_start(out=sc_sb, in_=sc_d.ap())

            def late_consts():
                # needed by phase E
                nc.gpsimd.dma_start(out=wout_sb, in_=wout_d.ap())
                nc.gpsimd.dma_start(out=g4_sb, in_=g4_d.ap())
                nc.gpsimd.dma_start(out=gb4_sb, in_=gb4_d.ap())
                nc.gpsimd.dma_start(out=be4_sb, in_=be4_d.ap())

            batch_seq = [bi for _ in range(repeat) for bi in range(BPC)]

            def phase_a_init(b, i_b=-1):
                xa = x_d.ap()[b].rearrange("(cc p) n -> p cc n", p=128)
                eq = eqp.tile([128, NSP], f32r, name="eq")
                ekt = ektp.tile([128, 32, 128], ctxdt, name="ekt")
                vta = vtap.tile([128, 32, VTW], ctxdt, name="vta")
                nc.gpsimd.dma_start(out=vta[:, :, 128:VTW], in_=vones_d.ap())
                recipd = rdp.tile([128, NSP], f32, name="recipd")
                return {"xa": xa, "eq": eq, "ekt": ekt, "vta": vta,
                        "recipd": recipd, "i_b": i_b}

            def phase_a_slice(S, j):
                xa, eq, ekt, vta, recipd = (
                    S["xa"], S["eq"], S["ekt"], S["vta"], S["recipd"]
                )
                if True:
                    sl = slice(j * SLICE, (j + 1) * SLICE)
                    if (S["i_b"], j) in pre_xs:
                        xs = pre_xs.pop((S["i_b"], j))
                    else:
                        xs = xp.tile([128, KC, SLICE], f32r, tag="xs", name="xs")
                        nc.sync.dma_start(out=xs, in_=xa[:, :, sl])

                    # q projection (standard layout) + exp
                    psq = pp.tile([128, SLICE], f32, tag="ps", name="psq")
                    for cc in range(KC):
                        nc.tensor.matmul(
                            psq[:, :],
                            r(wq_sb[:, cc, :]),
                            r(xs[:, cc, :]),
                            start=(cc == 0),
                            stop=(cc == KC - 1),
                        )
                    nc.scalar.activation(out=eq[:, sl], in_=psq[:, :], func=AF.Exp)

                    # softmax-over-d denominator, broadcast within head + recip
                    psd = pp.tile([128, SLICE], f32, tag="ps", name="psd")
                    nc.tensor.matmul(
                        psd[:, :], r(bones_sb[:, :]), r(eq[:, sl]), start=True, stop=True
                    )
                    nc.vector.reciprocal_approx_fast(out=recipd[:, sl], in_=psd[:, :])

                    # k,v transposed projections: x sub-slices as stationary operand
                    for h in range(2):
                        pskv = pp.tile([128, SLICE], f32, tag="ps", name="pskv")
                        for s_ in range(2):
                            isub = 2 * h + s_
                            for cc in range(KC):
                                nc.tensor.matmul(
                                    pskv[:, s_ * 256 : (s_ + 1) * 256],
                                    r(xs[:, cc, isub * 128 : (isub + 1) * 128]),
                                    r(wkv_sb[:, cc, :]),
                                    start=(cc == 0),
                                    stop=(cc == KC - 1),
                                )
                        kvv = pskv[:, :].rearrange("p (s o) -> p s o", s=2)
                        a0 = 4 * j + 2 * h
                        nc.scalar.activation(
                            out=ekt[:, a0 : a0 + 2, :], in_=kvv[:, :, 0:128], func=AF.Exp
                        )
                        nc.vector.tensor_copy(
                            out=vta[:, a0 : a0 + 2, 0:128], in_=kvv[:, :, 128:256]
                        )

            def phase_bcd(b, S):
                eq, ekt, vta, recipd = S["eq"], S["ekt"], S["vta"], S["recipd"]
                att = attp.tile([128, NSP], f32r, name="att")
                racc = smalls.tile([128, NSL], f32, name="racc")
                zacc = smalls.tile([128, NSL], f32, name="zacc")

                # context (accumulate over all 32 spatial chunks)
                psctx = ppctx.tile([128, SLICE], f32, name="psctx")
                for i in range(32):
                    nc.tensor.matmul(
                        psctx[:, 0:VTW],
                        ekt[:, i, :] if ctx_bf16 else r(ekt[:, i, :]),
                        vta[:, i, :] if ctx_bf16 else r(vta[:, i, :]),
                        start=(i == 0),
                        stop=(i == 31),
                    )
                inv_sk = smalls.tile([128, 1], f32, name="inv_sk")
                nc.vector.reciprocal_approx_fast(out=inv_sk, in_=psctx[:, 128:129])
                ctx2 = smalls.tile([128, HID], f32r, name="ctx2")
                nc.vector.scalar_tensor_tensor(
                    out=ctx2,
                    in0=psctx[:, 0:HID],
                    scalar=inv_sk,
                    in1=mask_sb[:, :],
                    op0=OP.mult,
                    op1=OP.mult,
                )

                # att = (ctx2_bd @ eq) * recipD; z = L^T att; accumulate stats
                for j in range(NSL):
                    sl = slice(j * SLICE, (j + 1) * SLICE)
                    pso = ppcd.tile([128, SLICE], f32, tag="pscd", name="pso")
                    nc.tensor.matmul(
                        pso[:, :], r(ctx2[:, :]), r(eq[:, sl]), start=True, stop=True
                    )
                    nc.vector.scalar_tensor_tensor(
                        out=att[:, sl],
                        in0=pso[:, :],
                        scalar=1.0,
                        in1=recipd[:, sl],
                        op0=OP.bypass,
                        op1=OP.mult,
                        accum_out=racc[:, j : j + 1],
                    )
                    psz = ppcd.tile([128, SLICE], f32, tag="pscd", name="psz")
                    nc.tensor.matmul(
                        psz[:, :], r(lmat_sb[:, :]), r(att[:, sl]), start=True, stop=True
                    )
                    if KZIP:
                        nc.scalar.activation(
                            out=psz[:, :], in_=psz[:, :], func=AF.Square,
                            accum_out=zacc[:, j : j + 1],
                        )
                    else:
                        zs = zp.tile([128, SLICE], f32, name="zs")
                        nc.scalar.activation(
                            out=zs, in_=psz[:, :], func=AF.Square,
                            accum_out=zacc[:, j : j + 1],
                        )

                r_sb = smalls.tile([128, 1], f32, name="r_sb")
                nc.vector.tensor_reduce(out=r_sb, in_=racc, axis=AX.X, op=OP.add)
                zred = smalls.tile([128, 1], f32, name="zred")
                nc.vector.tensor_reduce(out=zred, in_=zacc, axis=AX.X, op=OP.add)
                # duplicate into even-width f32r stationary operands
                rr = smalls.tile([128, 2], f32r, name="rr")
                nc.vector.tensor_copy(out=rr[:, 0:1], in_=r_sb)
                nc.vector.tensor_copy(out=rr[:, 1:2], in_=r_sb)
                zz = smalls.tile([128, 2], f32r, name="zz")
                nc.vector.tensor_copy(out=zz[:, 0:1], in_=zred)
                nc.vector.tensor_copy(out=zz[:, 1:2], in_=zred)

                # partition-0 dots: psA[0, 0]=<colsum_W,r>, psA[0, 1]=<W^T b, r>
                psA = ppcd.tile([128, SLICE], f32, tag="pscd", name="psA")
                nc.tensor.matmul(
                    psA[0:2, 0:4], r(rr[:, :]), r(u2_sb[:, :]), start=True, stop=True
                )
                # psB[0, 0] = sum_p zred = ||z||^2 total
                psB = ppcd.tile([128, SLICE], f32, tag="pscd", name="psB")
                nc.tensor.matmul(
                    psB[0:2, 0:4], r(zz[:, :]), r(ones_sb[:, :]), start=True, stop=True
                )

                # scalar chain on partition 0 (fused):
                st = stp.tile([1, 16], f32, name="st")
                inv_n = 1.0 / (C * NSP)
                nc.vector.scalar_tensor_tensor(
                    out=st[:, 2:3], in0=psA[0:1, 0:1], scalar=inv_n,
                    in1=sc_sb[:, 4:5], op0=OP.mult, op1=OP.add,
                )
                nc.vector.scalar_tensor_tensor(
                    out=st[:, 0:1], in0=psA[0:1, 1:2], scalar=2.0 * inv_n,
                    in1=sc_sb[:, 5:6], op0=OP.mult, op1=OP.add,
                )
                nc.vector.scalar_tensor_tensor(
                    out=st[:, 1:2], in0=psB[0:1, 0:1], scalar=inv_n,
                    in1=st[:, 0:1], op0=OP.mult, op1=OP.add,
                )
                nc.vector.tensor_mul(out=st[:, 3:4], in0=st[:, 2:3], in1=st[:, 2:3])
                nc.vector.tensor_sub(out=st[:, 4:5], in0=st[:, 1:2], in1=st[:, 3:4])
                # rstd = rsqrt(se), DVE-only: bit-trick seed + 3 Newton steps
                # (avoids Ln/Sqrt ACT table-set switches; se > 0 always)
                i32 = mybir.dt.int32
                nc.vector.tensor_scalar(
                    out=st[:, 6:7].bitcast(i32), in0=st[:, 4:5].bitcast(i32),
                    scalar1=1, scalar2=None, op0=OP.arith_shift_right,
                )
                nc.vector.tensor_scalar(
                    out=st[:, 6:7].bitcast(i32), in0=st[:, 6:7].bitcast(i32),
                    scalar1=-1, scalar2=0x5F3759DF,
                    op0=OP.mult, op1=OP.add,
                )
                for it in range(3):
                    dst = st[:, 8:9] if it == 2 else st[:, 6:7]
                    nc.vector.tensor_mul(out=st[:, 10:11], in0=st[:, 6:7], in1=st[:, 6:7])
                    nc.vector.tensor_mul(out=st[:, 10:11], in0=st[:, 10:11], in1=st[:, 4:5])
                    nc.vector.tensor_scalar(
                        out=st[:, 10:11], in0=st[:, 10:11],
                        scalar1=-0.5, scalar2=1.5, op0=OP.mult, op1=OP.add,
                    )
                    nc.vector.tensor_mul(out=dst, in0=st[:, 6:7], in1=st[:, 10:11])
                nc.vector.tensor_mul(out=st[:, 9:10], in0=st[:, 2:3], in1=st[:, 8:9])

                # broadcast (rstd, mu*rstd) across partitions via a K=1 matmul
                stb = stp.tile([1, 2], f32r, name="stb")
                nc.vector.tensor_copy(out=stb, in_=st[:, 8:10])
                psb = ppcd.tile([128, SLICE], f32, tag="pscd", name="psb")
                nc.tensor.matmul(
                    psb[:, 0:2], r(onesr_sb[:, :]), r(stb[:, :]), start=True, stop=True
                )
                rb = smalls.tile([128, 2], f32, name="rb")
                nc.vector.tensor_copy(out=rb, in_=psb[:, 0:2])

                # A = gamma*rstd ; Bc = gb*rstd - gamma*(mu*rstd) + beta
                a4 = smalls.tile([128, MC], f32, name="a4")
                nc.vector.tensor_scalar_mul(out=a4, in0=g4_sb[:, :], scalar1=rb[:, 0:1])
                b4 = smalls.tile([128, MC], f32, name="b4")
                nc.vector.tensor_scalar_mul(out=b4, in0=gb4_sb[:, :], scalar1=rb[:, 0:1])
                t4 = smalls.tile([128, MC], f32, name="t4")
                nc.vector.tensor_scalar_mul(out=t4, in0=g4_sb[:, :], scalar1=rb[:, 1:2])
                nc.vector.tensor_sub(out=b4, in0=b4, in1=t4)
                nc.vector.tensor_add(out=b4, in0=b4, in1=be4_sb[:, :])
                S["att"] = att
                S["a4"] = a4
                S["b4"] = b4

            def phase_e(b, S):
                att, a4, b4 = S["att"], S["a4"], S["b4"]
                oa = out_d.ap()[b].rearrange("(mm p) n -> p mm n", p=128)
                for j in range(NSL):
                    sl = slice(j * SLICE, (j + 1) * SLICE)
                    ys4 = yp.tile([128, MC, SLICE], f32, name="ys4")
                    for m in range(MC):
                        psy_pool, psy_tag = (ppy, "psy") if BUF_PSY else (ppcd, "pscd")
                        psy = psy_pool.tile([128, SLICE], f32, tag=psy_tag, name="psy")
                        nc.tensor.matmul(
                            psy[:, :],
                            r(wout_sb[:, m * 128 : (m + 1) * 128]),
                            r(att[:, sl]),
                            start=True,
                            stop=True,
                        )
                        if YSPLIT and (m % 2 == 1):
                            nc.vector.tensor_scalar(
                                out=ys4[:, m, :], in0=psy[:, :],
                                scalar1=a4[:, m : m + 1], scalar2=b4[:, m : m + 1],
                                op0=OP.mult, op1=OP.add,
                            )
                        else:
                            nc.scalar.activation(
                                out=ys4[:, m, :], in_=psy[:, :], func=AF.Identity,
                                bias=b4[:, m : m + 1], scale=a4[:, m : m + 1],
                            )
                    store_eng = [nc.sync, nc.gpsimd, nc.scalar][KSTORE]
                    store_eng.dma_start(out=oa[:, :, sl], in_=ys4)

            def phase_a(b, i_b=-1):
                S = phase_a_init(b, i_b)
                for j in range(NSL):
                    phase_a_slice(S, j)
                return S

            if KPIPE == 2:
                # interleave the BPC batches of each repeat block slice-by-slice
                for rep in range(repeat):
                    bs = batch_seq[rep * BPC : (rep + 1) * BPC]
                    Ss = [phase_a_init(b, rep * BPC + k) for k, b in enumerate(bs)]
                    mid_consts()
                    for j in range(NSL):
                        for S in Ss:
                            phase_a_slice(S, j)
                    late_consts()
                    for b, S in zip(bs, Ss):
                        phase_bcd(b, S)
                    for b, S in zip(bs, Ss):
                        phase_e(b, S)
            elif KPIPE in (1, 3, 4, 5, 6):
                states = []
                mid_consts()
                late_consts()
                for i_b, b in enumerate(batch_seq):
                    S = phase_a_init(b, i_b)
                    states.append(S)
                    cut = {3: 3, 4: 5, 5: 4, 6: 6}.get(KPIPE)
                    for j in range(NSL):
                        phase_a_slice(S, j)
                        if j == cut and i_b > 0:
                            phase_e(batch_seq[i_b - 1], states[i_b - 1])
                    if KPIPE == 1 and i_b > 0:
                        phase_e(batch_seq[i_b - 1], states[i_b - 1])
                    phase_bcd(b, S)
                phase_e(batch_seq[-1], states[-1])
            else:
                mid_consts()
                late_consts()
                for i_b, b in enumerate(batch_seq):
                    S = phase_a(b, i_b)
                    phase_bcd(b, S)
                    phase_e(b, S)

    nc.compile()
    return nc


def _host_consts(W_qkv, W_out, b_out, gamma, beta, ctx_bf16=None):
    if ctx_bf16 is None:
        ctx_bf16 = CTX_BF16
    W_qkv = np.asarray(W_qkv, np.float32)
    W_out = np.asarray(W_out, np.float32)
    b_out = np.asarray(b_out, np.float32)
    gamma = np.asarray(gamma, np.float32)
    beta = np.asarray(beta, np.float32)

    c = {}
    c["wq_t"] = np.ascontiguousarray(W_qkv[:HID].T)                 # [512, 128]
    c["wkv_t"] = np.ascontiguousarray(W_qkv[HID : 3 * HID].T)       # [512, 256]
    c["wout_t"] = np.ascontiguousarray(W_out.T)                     # [128, 512]
    blk = np.kron(np.eye(HEADS, dtype=np.float32), np.ones((DH, DH), np.float32))
    c["b_ones"] = blk                                               # [128, 128]
    c["mask_scale"] = (blk * SCALE).astype(np.float32)
    G = (W_out.astype(np.float64).T @ W_out.astype(np.float64))
    try:
        L = np.linalg.cholesky(G)
    except np.linalg.LinAlgError:
        w_ev, V = np.linalg.eigh(G)
        L = V @ np.diag(np.sqrt(np.clip(w_ev, 0.0, None)))
    c["lmat"] = L.astype(np.float32)                                # [128, 128]
    u2 = np.zeros((HID, 4), np.float32)
    u2[:, 0] = W_out.sum(axis=0)
    u2[:, 1] = W_out.T @ b_out
    c["u2"] = u2                                                    # [128, 4]
    c["ones_col"] = np.ones((HID, 4), np.float32)
    c["ones_row"] = np.ones((1, HID), np.float32)
    if ctx_bf16:
        import ml_dtypes
        c["vones"] = np.ones((128, 32, VTW - 128), ml_dtypes.bfloat16)
    else:
        c["vones"] = np.ones((128, 32, VTW - 128), np.float32)
    c["gamma4"] = np.ascontiguousarray(gamma.reshape(MC, 128).T)
    c["gb4"] = np.ascontiguousarray((gamma * b_out).reshape(MC, 128).T)
    c["beta4"] = np.ascontiguousarray(beta.reshape(MC, 128).T)
    ntot = float(C) * float(NSP)
    sc = np.zeros((1, 8), np.float32)
    c1 = NSP * float(b_out.astype(np.float64).sum())
    c2 = NSP * float((b_out.astype(np.float64) ** 2).sum())
    sc[0, 0] = c1
    sc[0, 1] = c2
    sc[0, 2] = EPS
    sc[0, 3] = 1.0 / ntot
    sc[0, 4] = c1 / ntot                         # C1*invN
    sc[0, 5] = c2 / ntot + EPS                   # C2*invN + eps
    c["sc"] = sc
    return c


def kernel(x, W_qkv, W_out, b_out, gamma, beta):
    from concourse.bass_utils import run_bass_kernel_spmd

    x = np.asarray(x, np.float32)
    assert x.shape == (B, C, HGT, WID)
    if "nc" not in _CACHE:
        _CACHE["nc"] = _build_nc()
    nc = _CACHE["nc"]

    consts = _host_consts(W_qkv, W_out, b_out, gamma, beta)
    xr = x.reshape(B, C, NSP)
    in_maps = []
    for ci in range(NCORES):
        m = {"x": np.ascontiguousarray(xr[ci * BPC : (ci + 1) * BPC])}
        m.update(consts)
        in_maps.append(m)

    res = run_bass_kernel_spmd(nc, in_maps, core_ids=list(range(NCORES)))
    out = np.concatenate([r_["out"] for r_ in res.results], axis=0)
    return np.ascontiguousarray(out.reshape(B, C, HGT, WID).astype(np.float32))

